# revision 1
# baseline (speedup 1.0000x reference)
"""Trainium2 Bass kernel for nn_MultiHeadSelfAttention (B=4, T=2048, C=768,
H=12, Dh=64; scores scaled by sqrt(Dh)=8).

Sharding (8 NeuronCores): core c -> batch b = c//2, head-group g = c%2
(6 of 12 heads). Each core runs full attention for its 6 heads over the
whole sequence of its batch and produces the partial projection product
y_heads @ W_proj[rows-of-those-heads]; the host sums the two partials per
batch (row-sharded W_proj all-reduce done on host) and stacks batches.

Device algorithm (per core), everything float32r (full PE speed, ~2^-13):
  xT = x[b].T is pre-transposed on host so no on-device transposes needed.
  QKV:   Q^T,K^T stored per head [64+aug rows, T]; V stored [s, head, 64+1]
         with a fused ones column (row-sum trick).
  Bias:  scoresT[s,t] = k_s.q_t - m_t computed via an augmented contraction
         Ktil=[K;1;0pad], Qtil=[Q;-(m+5);0pad] (K=128 padded - K=65 f32r
         matmuls run at half speed, K=128 at full speed). m_t is a coarse
         per-query max over keys 0:256 (computed on-device: small matmul +
         DVE negated max-reduce + PE transpose), so exp args stay in a safe
         fp32 window regardless of input data.
  Smax:  E = exp(8*scoresT) via ScalarE from PSUM, [128,1024] insts.
  AV:    out[65, t] accumulates Vtil^T E over the 16 key blocks; row 64 is
         the softmax denominator S[t]. Normalize = DVE reciprocal +
         GpSimd partition_broadcast + DVE multiply (division folded before
         the projection, which is per-head linear).
  Proj:  out = yT^T @ wp with K=128 tiles of the 384 head-concat rows.
"""
from contextlib import ExitStack

import numpy as np

import concourse.bacc as bacc
import concourse.mybir as mybir
import concourse.tile as tile
from concourse import bass_utils
from concourse.bass import ts

F32 = mybir.dt.float32
F32R = mybir.dt.float32r
EXP = mybir.ActivationFunctionType.Exp

B, T, C = 4, 2048, 768
NH = 6           # heads per core
D = 64
HG = NH * D      # 384
THALF = 512
SCALE = 8.0


def emit_mha(nc, tc, loop_k=None):
    if loop_k is not None:
        with tc.For_i(0, loop_k, 1):
            emit_mha(nc, tc, loop_k=None)
        return

    NP = NH // 2          # head pairs
    NC = C // 128         # qkv contraction tiles
    NS = T // 128         # key blocks
    NTH = T // THALF      # query spans
    NB = THALF // 512 if THALF >= 512 else 1
    NBW = min(512, THALF)
    PB = HG // 128        # proj contraction tiles

    xT_d = nc.dram_tensor("xT", [C, T], F32, kind="ExternalInput").ap()
    wq_d = nc.dram_tensor("wq", [C, HG], F32, kind="ExternalInput").ap()
    wk_d = nc.dram_tensor("wk", [C, HG], F32, kind="ExternalInput").ap()
    wv_d = nc.dram_tensor("wv", [C, HG], F32, kind="ExternalInput").ap()
    wp_d = nc.dram_tensor("wp", [HG, C], F32, kind="ExternalInput").ap()
    ident_d = nc.dram_tensor("ident", [128, 128], F32,
                             kind="ExternalInput").ap()
    out_d = nc.dram_tensor("out", [T, C], F32, kind="ExternalOutput").ap()

    ctx = ExitStack()
    persist = ctx.enter_context(tc.tile_pool(name="persist", bufs=1))
    qt_sb = persist.tile([128, NH, T], F32R, name="qt_sb")
    kt_sb = persist.tile([128, NH, T], F32R, name="kt_sb")
    v_sb = persist.tile([128, NS, NH, 65], F32R, name="v_sb")
    ident_sb = persist.tile([128, 128], F32R, name="ident_sb")

    nc.sync.dma_start(ident_sb, ident_d.bitcast(F32R))
    nc.vector.memset(v_sb[:, :, :, 64:65].bitcast(F32), 1.0)
    # zero padding rows so scores run as full-speed K=128 matmuls
    # (partition starts must be 0/32/64/96: zero [64:128] then set row 64)
    nc.vector.memset(kt_sb[64:128, :, :].bitcast(F32), 0.0)
    nc.vector.memset(kt_sb[64:65, :, :].bitcast(F32), 1.0)
    nc.vector.memset(qt_sb[64:128, :, :].bitcast(F32), 0.0)

    # ---------------- phase 1: QKV projections ----------------
    with tc.tile_pool(name="ph1", bufs=1) as ph1, \
         tc.tile_pool(name="qkv_ps", bufs=1, space="PSUM") as qkv_ps:
        xT_sb = ph1.tile([128, NC, T], F32R, name="xT_sb")
        wq_sb = ph1.tile([128, NC, HG], F32R, name="wq_sb")
        wk_sb = ph1.tile([128, NC, HG], F32R, name="wk_sb")
        wv_sb = ph1.tile([128, NC, HG], F32R, name="wv_sb")
        xT_r = xT_d.bitcast(F32R).rearrange("(n k) t -> k n t", k=128)
        for ci in range(NC):
            for tg in range(2):
                nc.sync.dma_start(
                    xT_sb[:, ci, ts(tg, T // 2)], xT_r[:, ci, ts(tg, T // 2)]
                )
        for w_sb, w_d in ((wq_sb, wq_d), (wk_sb, wk_d), (wv_sb, wv_d)):
            w_r = w_d.bitcast(F32R).rearrange("(n k) h -> k n h", k=128)
            for ci in range(NC):
                nc.sync.dma_start(w_sb[:, ci, :], w_r[:, ci, :])

        QKW = min(512, T)
        # V for all heads at once (N=HG>=256 avoids the f32r small-N penalty)
        for si in range(NS):
            ps = qkv_ps.tile([128, HG], F32, name="v_ps", bufs=2)
            for ci in range(NC):
                nc.tensor.matmul(
                    ps, xT_sb[:, ci, ts(si, 128)], wv_sb[:, ci, :],
                    start=(ci == 0), stop=(ci == NC - 1),
                )
            nc.vector.tensor_copy(
                v_sb[:, si, :, 0:64], ps.rearrange("s (h d) -> s h d", h=NH),
            )
        for p in range(NP):
            # Q^T, K^T for this pair -> per-head rows at partitions 0:64
            for w_sb, dst in ((wq_sb, qt_sb), (wk_sb, kt_sb)):
                for tb in range(T // QKW):
                    ps = qkv_ps.tile([128, QKW], F32, name="qk_ps", bufs=3)
                    for ci in range(NC):
                        nc.tensor.matmul(
                            ps, w_sb[:, ci, ts(p, 128)],
                            xT_sb[:, ci, ts(tb, QKW)],
                            start=(ci == 0), stop=(ci == NC - 1),
                        )
                    nc.vector.tensor_copy(
                        dst[0:64, 2 * p, ts(tb, QKW)], ps[0:64, :]
                    )
                    nc.vector.tensor_copy(
                        dst[0:64, 2 * p + 1, ts(tb, QKW)], ps[64:128, :]
                    )
            # coarse per-query max -> qt row 64 = -(m+5)
            for h in (2 * p, 2 * p + 1):
                for tb in range(T // 512):
                    pm = qkv_ps.tile([128, 4, 256], F32, name="pm", bufs=1)
                    for j in range(4):
                        nc.tensor.matmul(
                            pm[:, j, :],
                            qt_sb[0:64, h, ts(4 * tb + j, 128)],
                            kt_sb[0:64, h, 0:256],
                            start=True, stop=True,
                        )
                    mx = ph1.tile([128, 97], F32R, name="mx", bufs=4)
                    nc.vector.tensor_reduce(
                        mx[:, 0:97:32], pm, axis=mybir.AxisListType.X,
                        op=mybir.AluOpType.max, negate=True,
                    )
                    tr = qkv_ps.tile([97, 128], F32, name="tr", bufs=1)
                    nc.tensor.matmul(tr, mx, ident_sb, start=True, stop=True)
                    for j in range(4):
                        nc.vector.tensor_scalar_add(
                            qt_sb[64:65, h, ts(4 * tb + j, 128)],
                            tr[32 * j:32 * j + 1, :], -5.0
                        )

    # ---------------- phase 2: attention ----------------
    ph2 = ctx.enter_context(tc.tile_pool(name="ph2", bufs=1))
    yt_sb = ph2.tile([128, PB, T], F32R, name="yt_sb")
    wp_sb = ph2.tile([128, PB, C], F32R, name="wp_sb")
    wp_r = wp_d.bitcast(F32R).rearrange("(p k) c -> k p c", k=128)
    for pb in range(PB):
        nc.sync.dma_start(wp_sb[:, pb, :], wp_r[:, pb, :])

    with tc.tile_pool(name="norm", bufs=2) as norm, \
         tc.tile_pool(name="e_pool", bufs=3) as e_pool, \
         tc.tile_pool(name="sc_ps", bufs=2, space="PSUM") as sc_ps, \
         tc.tile_pool(name="av_ps", bufs=2, space="PSUM") as av_ps:
        for th in range(NTH):
            t0 = th * THALF
            for p in range(NP):
                av = [
                    av_ps.tile([65, THALF], F32, name=f"av{h}")
                    for h in (0, 1)
                ]
                for si in range(NS):
                    sc = sc_ps.tile([128, 2 * THALF], F32, name="sc")
                    for h in (0, 1):
                        for nb in range(NB):
                            nc.tensor.matmul(
                                sc[:, h * THALF + nb * NBW:
                                   h * THALF + (nb + 1) * NBW],
                                kt_sb[:, 2 * p + h, ts(si, 128)],
                                qt_sb[:, 2 * p + h,
                                      t0 + nb * NBW:t0 + (nb + 1) * NBW],
                                start=True, stop=True,
                            )
                    e_t = e_pool.tile([128, 2 * THALF], F32R, name="e_t")
                    nc.scalar.activation(e_t, sc, EXP, bias=0.0, scale=SCALE)
                    for h in (0, 1):
                        for nb in range(NB):
                            nc.tensor.matmul(
                                av[h][:, ts(nb, NBW)],
                                v_sb[:, si, 2 * p + h, :],
                                e_t[:, h * THALF + nb * NBW:
                                    h * THALF + (nb + 1) * NBW],
                                start=(si == 0), stop=(si == NS - 1),
                            )
                # normalize: yT_h = av[0:64] * (1 / av[64])
                for h in (0, 1):
                    r_row = norm.tile([1, THALF], F32, name="r_row")
                    nc.vector.reciprocal(r_row, av[h][64:65, :])
                    rb = norm.tile([64, THALF], F32, name="rb")
                    nc.gpsimd.partition_broadcast(rb, r_row)
                    hh = 2 * p + h
                    nc.vector.tensor_mul(
                        yt_sb[(hh % 2) * D:(hh % 2) * D + D, hh // 2,
                              t0:t0 + THALF],
                        av[h][0:64, :], rb,
                    )

    # ---------------- phase 3: projection ----------------
    with tc.tile_pool(name="out_pool", bufs=3) as out_pool, \
         tc.tile_pool(name="proj_ps", bufs=3, space="PSUM") as proj_ps:
        for tb in range(T // 128):
            po = proj_ps.tile([128, C], F32, name="po")
            for pb in range(PB):
                for nb2 in range(2):
                    n0, n1 = (0, 512) if nb2 == 0 else (512, C)
                    nc.tensor.matmul(
                        po[:, n0:n1],
                        yt_sb[:, pb, ts(tb, 128)],
                        wp_sb[:, pb, n0:n1],
                        start=(pb == 0), stop=(pb == PB - 1),
                    )
            ob = out_pool.tile([128, C], F32, name="ob")
            nc.vector.tensor_copy(ob, po)
            nc.sync.dma_start(out_d[ts(tb, 128), :], ob)
    ctx.close()


_compiled = None


def _get_compiled():
    global _compiled
    if _compiled is None:
        nc = bacc.Bacc("TRN2", target_bir_lowering=False, debug=False)
        with tile.TileContext(nc) as tc:
            emit_mha(nc, tc)
        nc.compile()
        _compiled = nc
    return _compiled


def make_in_maps(x, W_qkv, W_proj):
    ident = np.eye(128, dtype=np.float32)
    in_maps = []
    for c in range(8):
        b, g = c // 2, c % 2
        in_maps.append({
            "xT": np.ascontiguousarray(x[b].T),
            "wq": np.ascontiguousarray(W_qkv[:, g * HG:(g + 1) * HG]),
            "wk": np.ascontiguousarray(W_qkv[:, C + g * HG:C + (g + 1) * HG]),
            "wv": np.ascontiguousarray(
                W_qkv[:, 2 * C + g * HG:2 * C + (g + 1) * HG]),
            "wp": np.ascontiguousarray(W_proj[g * HG:(g + 1) * HG, :]),
            "ident": ident,
        })
    return in_maps


def kernel(x, W_qkv, W_proj):
    x = np.asarray(x, dtype=np.float32)
    W_qkv = np.asarray(W_qkv, dtype=np.float32)
    W_proj = np.asarray(W_proj, dtype=np.float32)
    nc = _get_compiled()
    res = bass_utils.run_bass_kernel_spmd(
        nc, make_in_maps(x, W_qkv, W_proj), core_ids=list(range(8))
    )
    out = np.zeros((B, T, C), dtype=np.float32)
    for c in range(8):
        out[c // 2] += res.results[c]["out"]
    return out

